# revision 10
# baseline (speedup 1.0000x reference)
"""GGNN (GatedGraphConv, L=5, F=128) on 8 TRN2 NeuronCores — Bass kernel.

Sharding: nodes padded to 50176 = 8 x 49 x 128; core c owns nodes
[c*6272,(c+1)*6272). State kept transposed in SBUF: hT [128, 6272] fp16.
Per layer: (A) m = h @ W_l per 128-node block -> DRAM shard (natural rows);
(B) AllGather shards -> m_full [50176,128] fp16; (C) per padded 128-edge tile
(sorted by dst block): indirect-DMA row gather of m_full[src], selection
matrix S (DVE is_equal vs iota), PE matmul msg.T @ S accumulated per dst
block in PSUM -> aggT; (D) GRU in transposed space (PE gates + ACT
sigmoid/tanh with fused per-partition biases + DVE elementwise); final relu +
linear -> out [1,6272]/core, host concat + trim.

Raw bass + Bacc, explicit semaphores (this toolchain allows one wait + one
update per instruction; extra waits are standalone wait_ge ops; DMA completion
sems are per-ring-slot, strictly serialized).
"""

import sys

sys.path.insert(0, "/opt/trn_rl_repo")

import numpy as np
from contextlib import ExitStack

import concourse.bass as bass
from concourse import bacc, mybir
from concourse.bass import IndirectOffsetOnAxis

AF = mybir.ActivationFunctionType

N_NODES = 50000
F = 128
import os as _os
L = int(_os.environ.get("GGNN_L", "5"))
P = 128
N_CORES = 8
NB = 49
NPC = NB * P            # 6272
N_PAD = N_CORES * NPC   # 50176
GB = int(_os.environ.get("GGNN_GB", "1"))     # tiles per indirect gather instruction
R = 8                   # msg ring slots (units of GB tiles)
PS_N = 4                # psum ring slots (agg/m)
WIN = 512
N_WIN = 13
WIN_W = [WIN] * 12 + [128]

DT = mybir.dt.float16
F32 = mybir.dt.float32


def _prep_edges(edge_index):
    src = np.asarray(edge_index[0], dtype=np.int64)
    dst = np.asarray(edge_index[1], dtype=np.int64)
    core = dst // NPC
    per_core = []
    kmax = 1
    for c in range(N_CORES):
        m = core == c
        s_c = src[m].astype(np.int32)
        d_c = (dst[m] - c * NPC).astype(np.int32)
        blk = d_c // P
        order = np.argsort(blk, kind="stable")
        s_c, d_c, blk = s_c[order], d_c[order], blk[order]
        counts = np.bincount(blk, minlength=NB)
        kmax = max(kmax, int(np.ceil(counts.max() / P)))
        per_core.append((s_c, d_c, counts))
    # tiles per block padded to kmax; total tiles padded to multiple of GB
    T = NB * kmax
    T_pad = ((T + GB - 1) // GB) * GB
    srcs, rels = [], []
    for c in range(N_CORES):
        s_c, d_c, counts = per_core[c]
        src_arr = np.zeros((T_pad * P,), np.int32)
        rel_arr = np.full((T_pad * P,), -1.0, np.float16)
        starts = np.concatenate([[0], np.cumsum(counts)])
        for b in range(NB):
            e0, e1 = int(starts[b]), int(starts[b + 1])
            n = e1 - e0
            o = b * kmax * P
            src_arr[o:o + n] = s_c[e0:e1]
            rel_arr[o:o + n] = (d_c[e0:e1] % P).astype(np.float16)
        srcs.append(np.ascontiguousarray(src_arr.reshape(T_pad, P).T))
        rels.append(np.ascontiguousarray(rel_arr.reshape(T_pad, P).T))
    return srcs, rels, T_pad, kmax


def _build(T, kmax):
    nc = bacc.Bacc("TRN2", target_bir_lowering=False)
    NBATCH = T // GB

    h0T_d = nc.dram_tensor("h0T", [P, NPC], DT, kind="ExternalInput")
    W_d = nc.dram_tensor("W_all", [P, L * F], DT, kind="ExternalInput")
    wih_d = nc.dram_tensor("w_ihT", [P, 3 * F], DT, kind="ExternalInput")
    whh_d = nc.dram_tensor("w_hhT", [P, 3 * F], DT, kind="ExternalInput")
    bias_d = nc.dram_tensor("bias", [P, 5], F32, kind="ExternalInput")
    lin_d = nc.dram_tensor("lin_wT", [P, 1], DT, kind="ExternalInput")
    idx_d = nc.dram_tensor("src_idx", [P, T], mybir.dt.int32, kind="ExternalInput")
    S_d = nc.dram_tensor("S_all", [P, T * P], DT, kind="ExternalInput")
    out_d = nc.dram_tensor("outT", [1, NPC], F32, kind="ExternalOutput")

    m_shard = nc.dram_tensor("m_shard", [NPC, F], DT)
    m_full = nc.dram_tensor("m_full", [N_PAD, F], DT, addr_space="Shared")

    ctx = ExitStack()
    sb = lambda n, s, d: ctx.enter_context(nc.sbuf_tensor(n, s, d))
    hT = sb("hT", [P, NPC], DT)
    aggT = sb("aggT", [P, NPC], DT)
    m_stage = sb("m_stage", [P, NPC], DT)
    idx_sb = sb("idx_sb", [P, T], mybir.dt.int32)
    SCH = 49           # S tiles per streamed chunk
    S_sb = sb("S_sb", [P, 2 * SCH * P], DT)
    W_sb = sb("W_sb", [P, L * F], DT)
    wih_sb = sb("wih_sb", [P, 3 * F], DT)
    whh_sb = sb("whh_sb", [P, 3 * F], DT)
    bias_sb = sb("bias_sb", [P, 5], F32)
    lin_sb = sb("lin_sb", [P, 1], DT)
    msg = sb("msg", [P, R * F], DT)
    tmp = {k: sb(f"t_{k}", [P, 2 * WIN], DT)
           for k in ("r", "z", "hnb", "inb", "npre", "n", "ru")}
    outT_sb = sb("outT_sb", [1, NPC], F32)


    ps_agg = ctx.enter_context(nc.psum_tensor("ps_agg", [P, PS_N * 512], F32))
    ps_gru = ctx.enter_context(nc.psum_tensor("ps_gru", [P, 4 * 512], F32))
    pr = lambda i, Wd: ps_gru[:, i * 512:i * 512 + Wd]

    sem = lambda n: ctx.enter_context(nc.semaphore(n))
    s_ld = sem("s_ld")
    s_g = [sem(f"s_g{i}") for i in range(R)]
    s_mm = sem("s_mm")
    s_dr = sem("s_dr")
    s_dma = sem("s_dma")
    s_cc = sem("s_cc")
    s_sd = [sem("s_sd0"), sem("s_sd1")]
    s_gate = sem("s_gate")
    s_dve = sem("s_dve")
    s_out = sem("s_out")

    n_mm = 0
    n_dr = 0
    n_gate = 0
    n_dve = 0
    n_dma = 0
    n_g = [0] * R
    mm_slot_free_cycle = 0     # mm count when last ring cycle fully consumed
    n_sd = [0, 0]              # S chunk DMAs per parity
    sch_mm_end = {}            # global chunk -> mm count when consumed
    sd_thresh = {}             # global chunk -> s_sd threshold
    psum_use = 0
    win_gate_end = []          # cumulative gate count at end of each D window
    win_dve_end = []

    nc.sync.dma_start(out=hT.ap(), in_=h0T_d[:, :]).then_inc(s_ld, 16)
    nc.sync.dma_start(out=idx_sb.ap(), in_=idx_d[:, :]).then_inc(s_ld, 16)
    nc.sync.dma_start(out=W_sb.ap(), in_=W_d[:, :]).then_inc(s_ld, 16)
    nc.sync.dma_start(out=wih_sb.ap(), in_=wih_d[:, :]).then_inc(s_ld, 16)
    nc.sync.dma_start(out=whh_sb.ap(), in_=whh_d[:, :]).then_inc(s_ld, 16)
    nc.sync.dma_start(out=bias_sb.ap(), in_=bias_d[:, :]).then_inc(s_ld, 16)
    nc.sync.dma_start(out=lin_sb.ap(), in_=lin_d[:, :]).then_inc(s_ld, 16)
    for eng in (nc.tensor, nc.vector, nc.scalar, nc.gpsimd):
        eng.wait_ge(s_ld, 7 * 16)

    bias_r = bias_sb[:, 0:1]
    bias_z = bias_sb[:, 1:2]
    bias_hn = bias_sb[:, 2:3]
    bias_in = bias_sb[:, 3:4]
    bias_lin = bias_sb[0:1, 4:5]

    for layer in range(L):
        # ---- A: m = h @ W_l ----
        if layer > 0:
            nc.tensor.wait_ge(s_dve, 2 * N_WIN * layer)   # h final
        nc.scalar.wait_ge(s_dma, 16 * n_dma)               # m_stage free
        for b in range(NB):
            slot = psum_use % PS_N
            if psum_use >= PS_N:
                nc.tensor.wait_ge(s_dr, psum_use - PS_N + 1)
            psum_use += 1
            nc.tensor.matmul(
                out=ps_agg[:, slot * 512: slot * 512 + F],
                lhsT=hT[:, b * P:(b + 1) * P],
                rhs=W_sb[:, layer * F:(layer + 1) * F],
                start=True, stop=True,
            ).then_inc(s_mm, 1)
            n_mm += 1
            nc.scalar.wait_ge(s_mm, n_mm)
            nc.scalar.copy(
                out=m_stage[:, b * P:(b + 1) * P],
                in_=ps_agg[:, slot * 512: slot * 512 + F],
            ).then_inc(s_dr, 1)
            n_dr += 1
        nc.sync.wait_ge(s_dr, n_dr)
        nc.sync.wait_ge(s_cc, layer)     # CC(l-1) done reading m_shard
        nc.sync.dma_start(
            out=m_shard.rearrange("(b p) f -> p b f", p=P),
            in_=m_stage.ap().rearrange("p (b f) -> p b f", f=F),
        ).then_inc(s_dma, 16)
        n_dma += 1

        # ---- B: AllGather ----
        for i in range(R):
            nc.gpsimd.wait_ge(s_g[i], 16 * n_g[i])   # prior gathers done w/ m_full
        nc.gpsimd.wait_ge(s_dma, 16 * n_dma)
        nc.gpsimd.collective_compute(
            "AllGather",
            mybir.AluOpType.bypass,
            replica_groups=[list(range(N_CORES))],
            ins=[m_shard.ap().opt()],
            outs=[m_full.ap().opt()],
        ).then_inc(s_cc, 1)
        nc.gpsimd.wait_ge(s_cc, layer + 1)

        # ---- C: gather + streamed-S segment matmul ----
        NCH = (NBATCH + SCH - 1) // SCH
        slot_b = 0

        def issue_s_chunk(ch):
            # DMA S chunk ch (this layer) into parity ch%2 buffer
            par = ch % 2
            gch = layer * NCH + ch
            if gch >= 2:
                nc.sync.wait_ge(s_mm, sch_mm_end[gch - 2])
            t0, t1 = ch * SCH, min((ch + 1) * SCH, NBATCH)
            nc.sync.dma_start(
                out=S_sb[:, par * SCH * P:par * SCH * P + (t1 - t0) * P],
                in_=S_d[:, t0 * P:t1 * P],
            ).then_inc(s_sd[par], 16)
            n_sd[par] += 1
            sd_thresh[gch] = 16 * n_sd[par]

        issue_s_chunk(0)
        if NCH > 1:
            issue_s_chunk(1)
        for ti in range(NBATCH):
            ring = ti % R
            ch = ti // SCH
            par = ch % 2
            if ti % SCH == 0:
                # PE: S chunk ch available?
                nc.tensor.wait_ge(s_sd[par], sd_thresh[layer * NCH + ch])
            if n_g[ring] > 0 and ring == 0:
                nc.gpsimd.wait_ge(s_mm, mm_slot_free_cycle)
            nc.gpsimd.indirect_dma_start(
                out=msg[:, ring * F:(ring + 1) * F],
                out_offset=None,
                in_=m_full[:],
                in_offset=IndirectOffsetOnAxis(
                    ap=idx_sb[:, ti:ti + 1], axis=0),
            ).then_inc(s_g[ring], 16)
            n_g[ring] += 1
            b = ti // kmax
            if b < NB:
                t_in_b = ti % kmax
                if t_in_b == 0:
                    slot_b = psum_use % PS_N
                    if psum_use >= PS_N:
                        nc.tensor.wait_ge(s_dr, psum_use - PS_N + 1)
                    psum_use += 1
                nc.tensor.wait_ge(s_g[ring], 16 * n_g[ring])
                nc.tensor.matmul(
                    out=ps_agg[:, slot_b * 512: slot_b * 512 + P],
                    lhsT=msg[:, ring * F:(ring + 1) * F],
                    rhs=S_sb[:, (par * SCH + (ti - ch * SCH)) * P:
                             (par * SCH + (ti - ch * SCH) + 1) * P],
                    start=(t_in_b == 0), stop=(t_in_b == kmax - 1),
                ).then_inc(s_mm, 1)
                n_mm += 1
                if t_in_b == kmax - 1:
                    nc.scalar.wait_ge(s_mm, n_mm)
                    nc.scalar.copy(
                        out=aggT[:, b * P:(b + 1) * P],
                        in_=ps_agg[:, slot_b * 512: slot_b * 512 + P],
                    ).then_inc(s_dr, 1)
                    n_dr += 1
            if ring == R - 1:
                mm_slot_free_cycle = n_mm
            if ti % SCH == SCH - 1 or ti == NBATCH - 1:
                sch_mm_end[layer * NCH + ch] = n_mm
                if ch + 2 < NCH:
                    issue_s_chunk(ch + 2)

        # ---- D: GRU over 13 windows ----
        drains_before_C = n_dr - NB
        for w in range(N_WIN):
            Wd = WIN_W[w]
            cw0 = w * WIN
            par = w % 2
            gw = len(win_gate_end)       # global window number
            agg_w = aggT[:, cw0:cw0 + Wd]
            h_w = hT[:, cw0:cw0 + Wd]
            nc.tensor.wait_ge(s_dr, drains_before_C + min(4 * (w + 1), NB))
            if gw >= 1:
                nc.tensor.wait_ge(s_gate, win_gate_end[gw - 1])  # psum_gru free
            nc.tensor.matmul(out=pr(0, Wd), lhsT=wih_sb[:, 0:F],
                             rhs=agg_w, start=True, stop=False)
            nc.tensor.matmul(out=pr(0, Wd), lhsT=whh_sb[:, 0:F],
                             rhs=h_w, start=False, stop=True).then_inc(s_mm, 1)
            n_mm += 1
            mm_r = n_mm
            nc.tensor.matmul(out=pr(1, Wd), lhsT=wih_sb[:, F:2 * F],
                             rhs=agg_w, start=True, stop=False)
            nc.tensor.matmul(out=pr(1, Wd), lhsT=whh_sb[:, F:2 * F],
                             rhs=h_w, start=False, stop=True).then_inc(s_mm, 1)
            n_mm += 1
            mm_z = n_mm
            nc.tensor.matmul(out=pr(2, Wd), lhsT=wih_sb[:, 2 * F:3 * F],
                             rhs=agg_w, start=True, stop=True).then_inc(s_mm, 1)
            n_mm += 1
            mm_in = n_mm
            nc.tensor.matmul(out=pr(3, Wd), lhsT=whh_sb[:, 2 * F:3 * F],
                             rhs=h_w, start=True, stop=True).then_inc(s_mm, 1)
            n_mm += 1
            mm_hn = n_mm

            t = lambda k: tmp[k][:, par * WIN: par * WIN + Wd]
            if gw >= 2:
                nc.scalar.wait_ge(s_dve, win_dve_end[gw - 2])  # temp parity free
            nc.scalar.wait_ge(s_mm, mm_r)
            nc.scalar.activation(t("r"), pr(0, Wd), AF.Sigmoid,
                                 bias=bias_r).then_inc(s_gate, 1)
            n_gate += 1
            nc.scalar.wait_ge(s_mm, mm_z)
            nc.scalar.activation(t("z"), pr(1, Wd), AF.Sigmoid,
                                 bias=bias_z).then_inc(s_gate, 1)
            n_gate += 1
            nc.scalar.wait_ge(s_mm, mm_hn)
            nc.scalar.activation(t("hnb"), pr(3, Wd), AF.Identity,
                                 bias=bias_hn).then_inc(s_gate, 1)
            n_gate += 1
            nc.scalar.wait_ge(s_mm, mm_in)
            nc.scalar.activation(t("inb"), pr(2, Wd), AF.Identity,
                                 bias=bias_in).then_inc(s_gate, 1)
            n_gate += 1
            nc.vector.wait_ge(s_gate, n_gate)
            nc.vector.tensor_mul(out=t("npre"), in0=t("r"), in1=t("hnb"))
            nc.vector.tensor_add(out=t("npre"), in0=t("npre"),
                                 in1=t("inb")).then_inc(s_dve, 1)
            n_dve += 1
            nc.scalar.wait_ge(s_dve, n_dve)
            nc.scalar.activation(t("n"), t("npre"), AF.Tanh).then_inc(s_gate, 1)
            n_gate += 1
            nc.vector.wait_ge(s_gate, n_gate)
            nc.vector.tensor_sub(out=t("hnb"), in0=h_w, in1=t("n"))
            nc.vector.tensor_mul(out=t("hnb"), in0=t("hnb"), in1=t("z"))
            nc.vector.tensor_add(out=h_w, in0=t("n"),
                                 in1=t("hnb")).then_inc(s_dve, 1)
            n_dve += 1
            win_gate_end.append(n_gate)
            win_dve_end.append(n_dve)

    # ---- E: out = relu(h) @ lin_w.T + lin_b (serialized, small) ----
    nc.scalar.wait_ge(s_dve, n_dve)
    for w in range(N_WIN):
        Wd = WIN_W[w]
        cw0 = w * WIN
        t_ru = tmp["ru"][:, 0:Wd]
        if w > 0:
            nc.scalar.wait_ge(s_mm, n_mm)    # previous matmul done reading ru
        nc.scalar.activation(t_ru, hT[:, cw0:cw0 + Wd],
                             AF.Relu).then_inc(s_gate, 1)
        n_gate += 1
        nc.tensor.wait_ge(s_gate, n_gate)
        if w >= 4:
            nc.tensor.wait_ge(s_gate, n_gate)  # (psum_o reuse covered below)
        nc.tensor.matmul(out=ps_gru[0:1, 0:Wd], lhsT=lin_sb[:, 0:1],
                         rhs=t_ru, start=True, stop=True).then_inc(s_mm, 1)
        n_mm += 1
        nc.scalar.wait_ge(s_mm, n_mm)
        nc.scalar.activation(outT_sb[0:1, cw0:cw0 + Wd], ps_gru[0:1, 0:Wd],
                             AF.Identity, bias=bias_lin).then_inc(s_gate, 1)
        n_gate += 1

    nc.sync.wait_ge(s_gate, n_gate)
    nc.sync.dma_start(out=out_d[:, :], in_=outT_sb.ap()).then_inc(s_out, 16)
    nc.sync.wait_ge(s_out, 16)
    ctx.close()
    nc.finalize()
    return nc


_CACHE = {}


def kernel(x, edge_index, weight, w_ih, w_hh, b_ih, b_hh, lin_w, lin_b):
    x = np.asarray(x, np.float32)
    weight = np.asarray(weight, np.float32)
    w_ih = np.asarray(w_ih, np.float32)
    w_hh = np.asarray(w_hh, np.float32)
    b_ih = np.asarray(b_ih, np.float32)
    b_hh = np.asarray(b_hh, np.float32)
    lin_w = np.asarray(lin_w, np.float32)
    lin_b = np.asarray(lin_b, np.float32)

    srcs, rels, T, kmax = _prep_edges(edge_index)
    key = (T, kmax)
    if key not in _CACHE:
        _CACHE[key] = _build(T, kmax)
    nc = _CACHE[key]

    W_all = np.concatenate([weight[l] for l in range(L)], axis=1).astype(np.float16)
    wihT = np.ascontiguousarray(w_ih.T).astype(np.float16)
    whhT = np.ascontiguousarray(w_hh.T).astype(np.float16)
    bias = np.zeros((P, 5), np.float32)
    bias[:, 0] = b_ih[0:F] + b_hh[0:F]
    bias[:, 1] = b_ih[F:2 * F] + b_hh[F:2 * F]
    bias[:, 2] = b_hh[2 * F:3 * F]
    bias[:, 3] = b_ih[2 * F:3 * F]
    bias[0, 4] = lin_b[0]
    linT = np.ascontiguousarray(lin_w.T).astype(np.float16)

    x_pad = np.zeros((N_PAD, F), np.float32)
    x_pad[:N_NODES] = x

    in_maps = []
    eye = np.eye(P, dtype=np.float16)
    for c in range(N_CORES):
        h0T = np.ascontiguousarray(x_pad[c * NPC:(c + 1) * NPC].T).astype(np.float16)
        # S_all[p, t*128 + d] = 1 if rel[p, t] == d else 0
        r = rels[c].astype(np.int32)                    # [P, T], -1 pads
        S_all = np.zeros((P, T * P), np.float16)
        valid = r >= 0
        pp, tt = np.nonzero(valid)
        S_flat = S_all.reshape(P, T, P)
        S_flat[pp, tt, r[pp, tt]] = 1.0
        in_maps.append({
            "h0T": h0T, "W_all": W_all, "w_ihT": wihT, "w_hhT": whhT,
            "bias": bias, "lin_wT": linT, "src_idx": srcs[c],
            "S_all": S_all,
        })

    from concourse.bass_utils import run_bass_kernel_spmd
    res = run_bass_kernel_spmd(nc, in_maps, list(range(N_CORES)))
    out = np.concatenate([res.results[c]["outT"][0] for c in range(N_CORES)])
    return out[:N_NODES, None].astype(np.float32)


if __name__ == "__main__":
    import jax
    import reference
    cpu = jax.devices("cpu")[0]
    with jax.default_device(cpu):
        inputs = {k: np.asarray(v) for k, v in reference.setup_inputs().items()}
        exp = np.asarray(reference.reference(**inputs))
    got = kernel(**inputs)
    err = np.abs(got - exp).max() / (np.abs(exp).max() + 1e-12)
    print("rel err:", err)


# revision 11
# speedup vs baseline: 2.5550x; 2.5550x over previous
"""GGNN (GatedGraphConv, L=5, F=128) on 8 TRN2 NeuronCores — Bass kernel.

Sharding: nodes padded to 50176 = 8 x 49 x 128; core c owns nodes
[c*6272,(c+1)*6272). State kept transposed in SBUF: hT [128, 6272] fp16.
Per layer: (A) m = h @ W_l per 128-node block -> DRAM shard (natural rows);
(B) AllGather shards -> m_full [50176,128] fp16; (C) per padded 128-edge tile
(sorted by dst block): indirect-DMA row gather of m_full[src], selection
matrix S (DVE is_equal vs iota), PE matmul msg.T @ S accumulated per dst
block in PSUM -> aggT; (D) GRU in transposed space (PE gates + ACT
sigmoid/tanh with fused per-partition biases + DVE elementwise); final relu +
linear -> out [1,6272]/core, host concat + trim.

Raw bass + Bacc, explicit semaphores (this toolchain allows one wait + one
update per instruction; extra waits are standalone wait_ge ops; DMA completion
sems are per-ring-slot, strictly serialized).
"""

import sys

sys.path.insert(0, "/opt/trn_rl_repo")

import numpy as np
from contextlib import ExitStack

import concourse.bass as bass
from concourse import bacc, mybir
from concourse.bass import IndirectOffsetOnAxis

AF = mybir.ActivationFunctionType

N_NODES = 50000
F = 128
import os as _os
L = int(_os.environ.get("GGNN_L", "5"))
P = 128
N_CORES = 8
NB = 49
NPC = NB * P            # 6272
N_PAD = N_CORES * NPC   # 50176
GB = int(_os.environ.get("GGNN_GB", "1"))     # tiles per indirect gather instruction
R = 8                   # msg ring slots (units of GB tiles)
PS_N = 4                # psum ring slots (agg/m)
WIN = 512
N_WIN = 13
WIN_W = [WIN] * 12 + [128]

DT = mybir.dt.float16
F32 = mybir.dt.float32


def _prep_edges(edge_index):
    src = np.asarray(edge_index[0], dtype=np.int64)
    dst = np.asarray(edge_index[1], dtype=np.int64)
    core = dst // NPC
    per_core = []
    kmax = 1
    for c in range(N_CORES):
        m = core == c
        s_c = src[m].astype(np.int32)
        d_c = (dst[m] - c * NPC).astype(np.int32)
        blk = d_c // P
        order = np.argsort(blk, kind="stable")
        s_c, d_c, blk = s_c[order], d_c[order], blk[order]
        counts = np.bincount(blk, minlength=NB)
        kmax = max(kmax, int(np.ceil(counts.max() / P)))
        per_core.append((s_c, d_c, counts))
    # tiles per block padded to kmax; total tiles padded to multiple of GB
    T = NB * kmax
    T_pad = ((T + GB - 1) // GB) * GB
    srcs, rels = [], []
    for c in range(N_CORES):
        s_c, d_c, counts = per_core[c]
        src_arr = np.zeros((T_pad * P,), np.int32)
        rel_arr = np.full((T_pad * P,), -1.0, np.float16)
        starts = np.concatenate([[0], np.cumsum(counts)])
        for b in range(NB):
            e0, e1 = int(starts[b]), int(starts[b + 1])
            n = e1 - e0
            o = b * kmax * P
            src_arr[o:o + n] = s_c[e0:e1]
            rel_arr[o:o + n] = (d_c[e0:e1] % P).astype(np.float16)
        srcs.append(np.ascontiguousarray(src_arr.reshape(T_pad, P).T))
        rels.append(np.ascontiguousarray(rel_arr.reshape(T_pad, P).T))
    return srcs, rels, T_pad, kmax


def _build(T, kmax):
    nc = bacc.Bacc("TRN2", target_bir_lowering=False)
    NBATCH = T // GB

    h0T_d = nc.dram_tensor("h0T", [P, NPC], DT, kind="ExternalInput")
    W_d = nc.dram_tensor("W_all", [P, L * F], DT, kind="ExternalInput")
    wih_d = nc.dram_tensor("w_ihT", [P, 3 * F], DT, kind="ExternalInput")
    whh_d = nc.dram_tensor("w_hhT", [P, 3 * F], DT, kind="ExternalInput")
    bias_d = nc.dram_tensor("bias", [P, 5], F32, kind="ExternalInput")
    lin_d = nc.dram_tensor("lin_wT", [P, 1], DT, kind="ExternalInput")
    idx_d = nc.dram_tensor("src_idx", [P, T], mybir.dt.int32, kind="ExternalInput")
    cf_d = nc.dram_tensor("cf", [P, T + P], DT, kind="ExternalInput")
    out_d = nc.dram_tensor("outT", [1, NPC], F32, kind="ExternalOutput")

    m_shard = nc.dram_tensor("m_shard", [NPC, F], DT)
    m_full = nc.dram_tensor("m_full", [N_PAD, F], DT, addr_space="Shared")

    ctx = ExitStack()
    sb = lambda n, s, d: ctx.enter_context(nc.sbuf_tensor(n, s, d))
    hT = sb("hT", [P, NPC], DT)
    aggT = sb("aggT", [P, NPC], DT)
    m_stage = sb("m_stage", [P, NPC], DT)
    idx_sb = sb("idx_sb", [P, T], mybir.dt.int32)
    cf_sb = sb("cf_sb", [P, T + P], DT)
    SCH = 49           # S tiles per built chunk
    S_sb = sb("S_sb", [P, 2 * SCH * P], DT)
    W_sb = sb("W_sb", [P, L * F], DT)
    wih_sb = sb("wih_sb", [P, 3 * F], DT)
    whh_sb = sb("whh_sb", [P, 3 * F], DT)
    bias_sb = sb("bias_sb", [P, 5], F32)
    lin_sb = sb("lin_sb", [P, 1], DT)
    msg = sb("msg", [P, R * F], DT)
    tmp = {k: sb(f"t_{k}", [P, 2 * WIN], DT)
           for k in ("r", "z", "hnb", "inb", "npre", "n", "ru")}
    outT_sb = sb("outT_sb", [1, NPC], F32)


    ps_agg = ctx.enter_context(nc.psum_tensor("ps_agg", [P, PS_N * 512], F32))
    ps_gru = ctx.enter_context(nc.psum_tensor("ps_gru", [P, 4 * 512], F32))
    pr = lambda i, Wd: ps_gru[:, i * 512:i * 512 + Wd]

    sem = lambda n: ctx.enter_context(nc.semaphore(n))
    s_ld = sem("s_ld")
    s_g = [sem(f"s_g{i}") for i in range(R)]
    s_mm = sem("s_mm")
    s_dr = sem("s_dr")
    s_dma = sem("s_dma")
    s_cc = sem("s_cc")
    s_sd = [sem("s_sd0"), sem("s_sd1")]
    s_gate = sem("s_gate")
    s_dve = sem("s_dve")
    s_out = sem("s_out")

    n_mm = 0
    n_dr = 0
    n_gate = 0
    n_dve = 0
    n_dma = 0
    n_g = [0] * R
    mm_slot_free_cycle = 0     # mm count when last ring cycle fully consumed
    n_sd = [0, 0]              # S chunk DMAs per parity
    sch_mm_end = {}            # global chunk -> mm count when consumed
    sd_thresh = {}             # global chunk -> s_sd threshold
    psum_use = 0
    win_gate_end = []          # cumulative gate count at end of each D window
    win_dve_end = []

    nc.sync.dma_start(out=hT.ap(), in_=h0T_d[:, :]).then_inc(s_ld, 16)
    nc.sync.dma_start(out=idx_sb.ap(), in_=idx_d[:, :]).then_inc(s_ld, 16)
    nc.sync.dma_start(out=cf_sb.ap(), in_=cf_d[:, :]).then_inc(s_ld, 16)
    nc.sync.dma_start(out=W_sb.ap(), in_=W_d[:, :]).then_inc(s_ld, 16)
    nc.sync.dma_start(out=wih_sb.ap(), in_=wih_d[:, :]).then_inc(s_ld, 16)
    nc.sync.dma_start(out=whh_sb.ap(), in_=whh_d[:, :]).then_inc(s_ld, 16)
    nc.sync.dma_start(out=bias_sb.ap(), in_=bias_d[:, :]).then_inc(s_ld, 16)
    nc.sync.dma_start(out=lin_sb.ap(), in_=lin_d[:, :]).then_inc(s_ld, 16)
    for eng in (nc.tensor, nc.vector, nc.scalar, nc.gpsimd):
        eng.wait_ge(s_ld, 8 * 16)

    bias_r = bias_sb[:, 0:1]
    bias_z = bias_sb[:, 1:2]
    bias_hn = bias_sb[:, 2:3]
    bias_in = bias_sb[:, 3:4]
    bias_lin = bias_sb[0:1, 4:5]

    for layer in range(L):
        # ---- A: m = h @ W_l ----
        if layer > 0:
            nc.tensor.wait_ge(s_dve, 2 * N_WIN * layer)   # h final
        nc.scalar.wait_ge(s_dma, 16 * n_dma)               # m_stage free
        for b in range(NB):
            slot = psum_use % PS_N
            if psum_use >= PS_N:
                nc.tensor.wait_ge(s_dr, psum_use - PS_N + 1)
            psum_use += 1
            nc.tensor.matmul(
                out=ps_agg[:, slot * 512: slot * 512 + F],
                lhsT=hT[:, b * P:(b + 1) * P],
                rhs=W_sb[:, layer * F:(layer + 1) * F],
                start=True, stop=True,
            ).then_inc(s_mm, 1)
            n_mm += 1
            nc.scalar.wait_ge(s_mm, n_mm)
            nc.scalar.copy(
                out=m_stage[:, b * P:(b + 1) * P],
                in_=ps_agg[:, slot * 512: slot * 512 + F],
            ).then_inc(s_dr, 1)
            n_dr += 1
        nc.sync.wait_ge(s_dr, n_dr)
        nc.sync.wait_ge(s_cc, layer)     # CC(l-1) done reading m_shard
        nc.sync.dma_start(
            out=m_shard.rearrange("(b p) f -> p b f", p=P),
            in_=m_stage.ap().rearrange("p (b f) -> p b f", f=F),
        ).then_inc(s_dma, 16)
        n_dma += 1

        # ---- B: AllGather ----
        for i in range(R):
            nc.gpsimd.wait_ge(s_g[i], 16 * n_g[i])   # prior gathers done w/ m_full
        nc.gpsimd.wait_ge(s_dma, 16 * n_dma)
        nc.gpsimd.collective_compute(
            "AllGather",
            mybir.AluOpType.bypass,
            replica_groups=[list(range(N_CORES))],
            ins=[m_shard.ap().opt()],
            outs=[m_full.ap().opt()],
        ).then_inc(s_cc, 1)
        nc.gpsimd.wait_ge(s_cc, layer + 1)

        # ---- C: gather + streamed-S segment matmul ----
        NCH = (NBATCH + SCH - 1) // SCH
        slot_b = 0

        def issue_s_chunk(ch):
            # build S chunk ch (this layer) into parity ch%2 buffer (DVE)
            par = ch % 2
            gch = layer * NCH + ch
            if gch >= 2:
                nc.vector.wait_ge(s_mm, sch_mm_end[gch - 2])
            t0, t1 = ch * SCH, min((ch + 1) * SCH, NBATCH)
            k = t1 - t0
            rel3 = cf_sb[:, t0:t1].rearrange(
                "p (t o) -> p t o", o=1).to_broadcast([P, k, P])
            iota3 = cf_sb[:, T:T + P].rearrange(
                "p (o d) -> p o d", o=1).to_broadcast([P, k, P])
            nc.vector.tensor_tensor(
                out=S_sb[:, par * SCH * P:par * SCH * P + k * P].rearrange(
                    "p (t d) -> p t d", d=P),
                in0=rel3, in1=iota3, op=mybir.AluOpType.is_equal,
            ).then_inc(s_sd[par], 1)
            n_sd[par] += 1
            sd_thresh[gch] = n_sd[par]

        issue_s_chunk(0)
        if NCH > 1:
            issue_s_chunk(1)
        for ti in range(NBATCH):
            ring = ti % R
            ch = ti // SCH
            par = ch % 2
            if ti % SCH == 0:
                # PE: S chunk ch available?
                nc.tensor.wait_ge(s_sd[par], sd_thresh[layer * NCH + ch])  # S chunk ready
            if n_g[ring] > 0 and ring == 0:
                nc.gpsimd.wait_ge(s_mm, mm_slot_free_cycle)
            nc.gpsimd.indirect_dma_start(
                out=msg[:, ring * F:(ring + 1) * F],
                out_offset=None,
                in_=m_full[:],
                in_offset=IndirectOffsetOnAxis(
                    ap=idx_sb[:, ti:ti + 1], axis=0),
            ).then_inc(s_g[ring], 16)
            n_g[ring] += 1
            b = ti // kmax
            if b < NB:
                t_in_b = ti % kmax
                if t_in_b == 0:
                    slot_b = psum_use % PS_N
                    if psum_use >= PS_N:
                        nc.tensor.wait_ge(s_dr, psum_use - PS_N + 1)
                    psum_use += 1
                nc.tensor.wait_ge(s_g[ring], 16 * n_g[ring])
                nc.tensor.matmul(
                    out=ps_agg[:, slot_b * 512: slot_b * 512 + P],
                    lhsT=msg[:, ring * F:(ring + 1) * F],
                    rhs=S_sb[:, (par * SCH + (ti - ch * SCH)) * P:
                             (par * SCH + (ti - ch * SCH) + 1) * P],
                    start=(t_in_b == 0), stop=(t_in_b == kmax - 1),
                ).then_inc(s_mm, 1)
                n_mm += 1
                if t_in_b == kmax - 1:
                    nc.scalar.wait_ge(s_mm, n_mm)
                    nc.scalar.copy(
                        out=aggT[:, b * P:(b + 1) * P],
                        in_=ps_agg[:, slot_b * 512: slot_b * 512 + P],
                    ).then_inc(s_dr, 1)
                    n_dr += 1
            if ring == R - 1:
                mm_slot_free_cycle = n_mm
            if ti % SCH == SCH - 1 or ti == NBATCH - 1:
                sch_mm_end[layer * NCH + ch] = n_mm
                if ch + 2 < NCH:
                    issue_s_chunk(ch + 2)

        # ---- D: GRU over 13 windows ----
        drains_before_C = n_dr - NB
        for w in range(N_WIN):
            Wd = WIN_W[w]
            cw0 = w * WIN
            par = w % 2
            gw = len(win_gate_end)       # global window number
            agg_w = aggT[:, cw0:cw0 + Wd]
            h_w = hT[:, cw0:cw0 + Wd]
            nc.tensor.wait_ge(s_dr, drains_before_C + min(4 * (w + 1), NB))
            if gw >= 1:
                nc.tensor.wait_ge(s_gate, win_gate_end[gw - 1])  # psum_gru free
            nc.tensor.matmul(out=pr(0, Wd), lhsT=wih_sb[:, 0:F],
                             rhs=agg_w, start=True, stop=False)
            nc.tensor.matmul(out=pr(0, Wd), lhsT=whh_sb[:, 0:F],
                             rhs=h_w, start=False, stop=True).then_inc(s_mm, 1)
            n_mm += 1
            mm_r = n_mm
            nc.tensor.matmul(out=pr(1, Wd), lhsT=wih_sb[:, F:2 * F],
                             rhs=agg_w, start=True, stop=False)
            nc.tensor.matmul(out=pr(1, Wd), lhsT=whh_sb[:, F:2 * F],
                             rhs=h_w, start=False, stop=True).then_inc(s_mm, 1)
            n_mm += 1
            mm_z = n_mm
            nc.tensor.matmul(out=pr(2, Wd), lhsT=wih_sb[:, 2 * F:3 * F],
                             rhs=agg_w, start=True, stop=True).then_inc(s_mm, 1)
            n_mm += 1
            mm_in = n_mm
            nc.tensor.matmul(out=pr(3, Wd), lhsT=whh_sb[:, 2 * F:3 * F],
                             rhs=h_w, start=True, stop=True).then_inc(s_mm, 1)
            n_mm += 1
            mm_hn = n_mm

            t = lambda k: tmp[k][:, par * WIN: par * WIN + Wd]
            if gw >= 2:
                nc.scalar.wait_ge(s_dve, win_dve_end[gw - 2])  # temp parity free
            nc.scalar.wait_ge(s_mm, mm_r)
            nc.scalar.activation(t("r"), pr(0, Wd), AF.Sigmoid,
                                 bias=bias_r).then_inc(s_gate, 1)
            n_gate += 1
            nc.scalar.wait_ge(s_mm, mm_z)
            nc.scalar.activation(t("z"), pr(1, Wd), AF.Sigmoid,
                                 bias=bias_z).then_inc(s_gate, 1)
            n_gate += 1
            nc.scalar.wait_ge(s_mm, mm_hn)
            nc.scalar.activation(t("hnb"), pr(3, Wd), AF.Identity,
                                 bias=bias_hn).then_inc(s_gate, 1)
            n_gate += 1
            nc.scalar.wait_ge(s_mm, mm_in)
            nc.scalar.activation(t("inb"), pr(2, Wd), AF.Identity,
                                 bias=bias_in).then_inc(s_gate, 1)
            n_gate += 1
            nc.vector.wait_ge(s_gate, n_gate)
            nc.vector.tensor_mul(out=t("npre"), in0=t("r"), in1=t("hnb"))
            nc.vector.tensor_add(out=t("npre"), in0=t("npre"),
                                 in1=t("inb")).then_inc(s_dve, 1)
            n_dve += 1
            nc.scalar.wait_ge(s_dve, n_dve)
            nc.scalar.activation(t("n"), t("npre"), AF.Tanh).then_inc(s_gate, 1)
            n_gate += 1
            nc.vector.wait_ge(s_gate, n_gate)
            nc.vector.tensor_sub(out=t("hnb"), in0=h_w, in1=t("n"))
            nc.vector.tensor_mul(out=t("hnb"), in0=t("hnb"), in1=t("z"))
            nc.vector.tensor_add(out=h_w, in0=t("n"),
                                 in1=t("hnb")).then_inc(s_dve, 1)
            n_dve += 1
            win_gate_end.append(n_gate)
            win_dve_end.append(n_dve)

    # ---- E: out = relu(h) @ lin_w.T + lin_b (serialized, small) ----
    nc.scalar.wait_ge(s_dve, n_dve)
    for w in range(N_WIN):
        Wd = WIN_W[w]
        cw0 = w * WIN
        t_ru = tmp["ru"][:, 0:Wd]
        if w > 0:
            nc.scalar.wait_ge(s_mm, n_mm)    # previous matmul done reading ru
        nc.scalar.activation(t_ru, hT[:, cw0:cw0 + Wd],
                             AF.Relu).then_inc(s_gate, 1)
        n_gate += 1
        nc.tensor.wait_ge(s_gate, n_gate)
        if w >= 4:
            nc.tensor.wait_ge(s_gate, n_gate)  # (psum_o reuse covered below)
        nc.tensor.matmul(out=ps_gru[0:1, 0:Wd], lhsT=lin_sb[:, 0:1],
                         rhs=t_ru, start=True, stop=True).then_inc(s_mm, 1)
        n_mm += 1
        nc.scalar.wait_ge(s_mm, n_mm)
        nc.scalar.activation(outT_sb[0:1, cw0:cw0 + Wd], ps_gru[0:1, 0:Wd],
                             AF.Identity, bias=bias_lin).then_inc(s_gate, 1)
        n_gate += 1

    nc.sync.wait_ge(s_gate, n_gate)
    nc.sync.dma_start(out=out_d[:, :], in_=outT_sb.ap()).then_inc(s_out, 16)
    nc.sync.wait_ge(s_out, 16)
    ctx.close()
    nc.finalize()
    return nc


_CACHE = {}


def kernel(x, edge_index, weight, w_ih, w_hh, b_ih, b_hh, lin_w, lin_b):
    x = np.asarray(x, np.float32)
    weight = np.asarray(weight, np.float32)
    w_ih = np.asarray(w_ih, np.float32)
    w_hh = np.asarray(w_hh, np.float32)
    b_ih = np.asarray(b_ih, np.float32)
    b_hh = np.asarray(b_hh, np.float32)
    lin_w = np.asarray(lin_w, np.float32)
    lin_b = np.asarray(lin_b, np.float32)

    srcs, rels, T, kmax = _prep_edges(edge_index)
    key = (T, kmax)
    if key not in _CACHE:
        _CACHE[key] = _build(T, kmax)
    nc = _CACHE[key]

    W_all = np.concatenate([weight[l] for l in range(L)], axis=1).astype(np.float16)
    wihT = np.ascontiguousarray(w_ih.T).astype(np.float16)
    whhT = np.ascontiguousarray(w_hh.T).astype(np.float16)
    bias = np.zeros((P, 5), np.float32)
    bias[:, 0] = b_ih[0:F] + b_hh[0:F]
    bias[:, 1] = b_ih[F:2 * F] + b_hh[F:2 * F]
    bias[:, 2] = b_hh[2 * F:3 * F]
    bias[:, 3] = b_ih[2 * F:3 * F]
    bias[0, 4] = lin_b[0]
    linT = np.ascontiguousarray(lin_w.T).astype(np.float16)

    x_pad = np.zeros((N_PAD, F), np.float32)
    x_pad[:N_NODES] = x

    iota = np.broadcast_to(np.arange(P, dtype=np.float16), (P, P))
    in_maps = []
    for c in range(N_CORES):
        h0T = np.ascontiguousarray(x_pad[c * NPC:(c + 1) * NPC].T).astype(np.float16)
        cf = np.concatenate([rels[c], iota], axis=1).astype(np.float16)
        in_maps.append({
            "h0T": h0T, "W_all": W_all, "w_ihT": wihT, "w_hhT": whhT,
            "bias": bias, "lin_wT": linT, "src_idx": srcs[c], "cf": cf,
        })

    from concourse.bass_utils import run_bass_kernel_spmd
    res = run_bass_kernel_spmd(nc, in_maps, list(range(N_CORES)))
    out = np.concatenate([res.results[c]["outT"][0] for c in range(N_CORES)])
    return out[:N_NODES, None].astype(np.float32)


if __name__ == "__main__":
    import jax
    import reference
    cpu = jax.devices("cpu")[0]
    with jax.default_device(cpu):
        inputs = {k: np.asarray(v) for k, v in reference.setup_inputs().items()}
        exp = np.asarray(reference.reference(**inputs))
    got = kernel(**inputs)
    err = np.abs(got - exp).max() / (np.abs(exp).max() + 1e-12)
    print("rel err:", err)
